# revision 18
# baseline (speedup 1.0000x reference)
"""CAM-module (channel attention) Trainium2 kernel.

Problem: B=4 samples, C=64, H=W=256 (N=65536 px). concat(rgb,hsv,lab) ->
X [192, N] per sample; q/k/v = 1x1-conv projections (W [64,192] + bias);
energy = q @ k^T * C^-0.5 -> softmax over last dim -> out = att @ v.

Sharding: 8 cores = 4 samples x 2 spatial halves (32768 px each). Each
core computes a partial energy over its half; a 16 KiB pairwise AllReduce
([[0,1],[2,3],[4,5],[6,7]]) completes the C x C energy, then each core
computes out for its own half. HBM traffic is the 32 MiB/core minimum.

Precision: the softmax logits have std ~850, so energy needs ~fp32
accuracy. Projections run as 3-pass fp16 hi/lo splits (X = Xh + Xl,
W = Wh + Wl; passes Xh@Wh + Xh@Wl + Xl@Wh, fp32 PSUM accumulate), which
keeps the dropped term at ~2^-22. The energy matmul itself runs in true
fp32 (4 cyc/row, tiny N). v uses Wh@(Xh+Xl); out = fp16(att) @ (vh+vl)
via a stacked [vh; vl] tile so both halves go through one matmul.
Measured vs fp64: absmax ~2e-3 == the fp32 reference's own envelope.

Biases fold in exactly via an appended ones-row on X (lab chunk becomes
65 partitions) and bias rows on the weight chunks (hi+lo).
"""

import sys
import numpy as np

if '/opt/trn_rl_repo' not in sys.path:
    sys.path.insert(0, '/opt/trn_rl_repo')

B, C, H, W = 4, 64, 256, 256
N = H * W                 # 65536 px per sample
NHALF = N // 2            # 32768 px per core
PX = 2048                 # streaming tile (px)
NIT = NHALF // PX         # 16
SUB = 128                 # qkT subtile (px) = matmul M
NSUB = PX // SUB          # 16
VC = 512                  # v / out chunk (px) = matmul N
NVC = PX // VC            # 4
NCORES = 8

_CACHE = {}


def _build_bass(single_core=False):
    import concourse.bacc as bacc
    import concourse.mybir as mybir
    from concourse import tile

    F32 = mybir.dt.float32
    F16 = mybir.dt.float16
    Exp = mybir.ActivationFunctionType.Exp

    nc = bacc.Bacc("TRN2", target_bir_lowering=False, debug=False,
                   enable_asserts=False,
                   num_devices=1 if single_core else NCORES)

    xr_d = nc.dram_tensor("x_rgb", [64, NHALF], F32, kind="ExternalInput").ap()
    xs_d = nc.dram_tensor("x_hsv", [64, NHALF], F32, kind="ExternalInput").ap()
    xb_d = nc.dram_tensor("x_lab", [64, NHALF], F32, kind="ExternalInput").ap()
    wqkh0_d = nc.dram_tensor("wqkh0", [128, 128], F16, kind="ExternalInput").ap()
    wqkh1_d = nc.dram_tensor("wqkh1", [65, 128], F16, kind="ExternalInput").ap()
    wqkl0_d = nc.dram_tensor("wqkl0", [128, 128], F16, kind="ExternalInput").ap()
    wqkl1_d = nc.dram_tensor("wqkl1", [65, 128], F16, kind="ExternalInput").ap()
    wvh0_d = nc.dram_tensor("wvh0", [128, 64], F16, kind="ExternalInput").ap()
    wvh1_d = nc.dram_tensor("wvh1", [65, 64], F16, kind="ExternalInput").ap()
    ident_d = nc.dram_tensor("ident", [64, 64], F32, kind="ExternalInput").ap()
    out_d = nc.dram_tensor("out", [64, NHALF], F32, kind="ExternalOutput").ap()

    with tile.TileContext(nc) as tc:
        with tc.tile_pool(name="const", bufs=1) as const, \
             tc.tile_pool(name="stream", bufs=3) as stream, \
             tc.tile_pool(name="qk", bufs=2) as qkpool, \
             tc.tile_pool(name="qkps", bufs=2, space="PSUM") as qkps, \
             tc.tile_pool(name="vps", bufs=2, space="PSUM") as vps, \
             tc.tile_pool(name="eps", bufs=1, space="PSUM") as eps, \
             tc.tile_pool(name="dram", bufs=1, space="DRAM") as dram:

            wqkh0 = const.tile([128, 128], F16)
            wqkh1 = const.tile([65, 128], F16)
            wqkl0 = const.tile([128, 128], F16)
            wqkl1 = const.tile([65, 128], F16)
            wvh0 = const.tile([128, 64], F16)
            wvh1 = const.tile([65, 64], F16)
            ident = const.tile([64, 64], F32)
            nc.sync.dma_start(wqkh0[:], wqkh0_d[:])
            nc.sync.dma_start(wqkh1[:], wqkh1_d[:])
            nc.sync.dma_start(wqkl0[:], wqkl0_d[:])
            nc.sync.dma_start(wqkl1[:], wqkl1_d[:])
            nc.sync.dma_start(wvh0[:], wvh0_d[:])
            nc.sync.dma_start(wvh1[:], wvh1_d[:])
            nc.sync.dma_start(ident[:], ident_d[:])

            vhl = const.tile([128, NHALF], F16)   # [vh; vl] stacked
            # 4 per-sample energy accumulators packed in one PSUM bank
            ep = eps.tile([64, 4 * 64], F32)

            NIT_S = NIT // B          # streaming tiles per sample (4)

            def sample_epilogue(s):
                # partial energy -> 8-way AllReduce -> softmax -> attT2[s]
                e_sb = const.tile([64, 64], F32, tag=f"e_sb{s}")
                nc.scalar.copy(e_sb[:], ep[:, s * 64:(s + 1) * 64])
                bi = dram.tile([64, 64], F32, tag=f"bi{s}")
                bo = dram.tile([64, 64], F32, tag=f"bo{s}")
                nc.gpsimd.dma_start(bi[:], e_sb[:])
                if single_core:
                    nc.gpsimd.dma_start(bo[:], bi[:])
                else:
                    nc.gpsimd.collective_compute(
                        "AllReduce", mybir.AluOpType.add,
                        replica_groups=[list(range(NCORES))],
                        ins=[bi.opt()], outs=[bo.opt()],
                    )
                e2 = const.tile([64, 64], F32, tag=f"e2{s}")
                nc.gpsimd.dma_start(e2[:], bo[:])
                m = const.tile([64, 1], F32, tag=f"m{s}")
                nc.vector.reduce_max(m[:], e2[:], axis=mybir.AxisListType.X)
                mb = const.tile([64, 1], F32, tag=f"mb{s}")
                nc.vector.tensor_scalar_mul(mb[:], m[:], -0.125)
                attu = const.tile([64, 64], F32, tag=f"attu{s}")
                ssum = const.tile([64, 1], F32, tag=f"ssum{s}")
                nc.scalar.activation(attu[:], e2[:], Exp, bias=mb[:], scale=0.125,
                                     accum_out=ssum[:])
                r = const.tile([64, 1], F32, tag=f"r{s}")
                nc.vector.reciprocal(r[:], ssum[:])
                att = const.tile([64, 64], F32, tag=f"att{s}")
                nc.vector.tensor_scalar_mul(att[:], attu[:], r[:])
                atp = eps.tile([64, 64], F32, tag="atp")
                nc.tensor.transpose(atp[:], att[:], ident[:])
                attT2 = const.tile([128, 64], F16, tag=f"attT2{s}")
                nc.scalar.copy(attT2[0:64, :], atp[:])
                nc.scalar.copy(attT2[64:128, :], atp[:])
                return attT2

            def sample_out_phase(s, attT2):
                # out = att @ (vh + vl) for this sample's 8192 px
                out_sb = None
                base_px = s * (NHALF // B)
                for og in range(NHALF // B // (2 * VC)):
                    op = vps.tile([64, 2 * VC], F32, tag="vp")
                    for h in range(2):
                        lo = base_px + (og * 2 + h) * VC
                        nc.tensor.matmul(op[:, h * VC:(h + 1) * VC], attT2[:],
                                         vhl[:, lo:lo + VC], start=True, stop=True)
                    w2 = og % 2
                    if w2 == 0:
                        out_sb = stream.tile([64, PX], F32, tag="out_sb")
                    base = w2 * 2 * VC
                    nc.scalar.copy(out_sb[:, base:base + VC], op[:, 0:VC])
                    nc.vector.tensor_copy(out_sb[:, base + VC:base + 2 * VC],
                                          op[:, VC:2 * VC])
                    if w2 == 1:
                        g = base_px + (og - 1) * 2 * VC
                        nc.sync.dma_start(out_d[:, g:g + PX], out_sb[:])

            for it in range(NIT):
                s_cur = it // NIT_S   # sample this tile belongs to
                sl = slice(it * PX, (it + 1) * PX)
                x0_32 = stream.tile([128, PX], F32, tag="x0_32")
                nc.sync.dma_start(x0_32[0:64, :], xr_d[:, sl])
                nc.sync.dma_start(x0_32[64:128, :], xs_d[:, sl])
                x1_32 = stream.tile([64, PX], F32, tag="x1_32")
                nc.sync.dma_start(x1_32[:], xb_d[:, sl])

                x0h = stream.tile([128, PX], F16, tag="x0h")
                nc.scalar.copy(x0h[:], x0_32[:])
                x0l = stream.tile([128, PX], F16, tag="x0l")
                nc.vector.tensor_sub(x0l[:], x0_32[:], x0h[:])
                x1h = stream.tile([65, PX], F16, tag="x1h")
                nc.scalar.copy(x1h[0:64, :], x1_32[:])
                x1l = stream.tile([65, PX], F16, tag="x1l")
                nc.vector.tensor_sub(x1l[0:64, :], x1_32[:], x1h[0:64, :])
                if it < 3:
                    # ones/zeros rows live in the 3 round-robin pool slots;
                    # later iterations reuse them untouched
                    nc.gpsimd.memset(x1h[64:65, :], 1.0)
                    nc.gpsimd.memset(x1l[64:65, :], 0.0)

                for vg in range(NVC // 2):   # v: 1-pass, 2 chunks per PSUM tile
                    vp = vps.tile([64, 2 * VC], F32, tag="vp")
                    for h in range(2):
                        vc = vg * 2 + h
                        vsl = slice(vc * VC, (vc + 1) * VC)
                        psl = slice(h * VC, (h + 1) * VC)
                        nc.tensor.matmul(vp[:, psl], wvh0[:], x0h[:, vsl],
                                         start=True, stop=False)
                        nc.tensor.matmul(vp[:, psl], wvh1[:], x1h[:, vsl],
                                         start=False, stop=True)
                    gsl = slice(it * PX + vg * 2 * VC, it * PX + (vg + 1) * 2 * VC)
                    nc.scalar.copy(vhl[0:64, gsl], vp[:])
                    nc.vector.tensor_sub(vhl[64:128, gsl], vp[:], vhl[0:64, gsl])

                for grp in range(NSUB // 4):   # qkT: 4 subtiles per PSUM bank
                    qkp = qkps.tile([128, 512], F32, tag="qkp")
                    for s4 in range(4):
                        sb = grp * 4 + s4
                        ssl = slice(sb * SUB, (sb + 1) * SUB)
                        osl = slice(s4 * 128, (s4 + 1) * 128)
                        nc.tensor.matmul(qkp[:, osl], x0h[:, ssl], wqkh0[:], start=True, stop=False)
                        nc.tensor.matmul(qkp[:, osl], x0h[:, ssl], wqkl0[:], start=False, stop=False)
                        nc.tensor.matmul(qkp[:, osl], x0l[:, ssl], wqkh0[:], start=False, stop=False)
                        nc.tensor.matmul(qkp[:, osl], x1h[:, ssl], wqkh1[:], start=False, stop=False)
                        nc.tensor.matmul(qkp[:, osl], x1h[:, ssl], wqkl1[:], start=False, stop=False)
                        nc.tensor.matmul(qkp[:, osl], x1l[:, ssl], wqkh1[:], start=False, stop=True)
                    qk_sb = qkpool.tile([128, 512], F32, tag="qk_sb")
                    nc.scalar.copy(qk_sb[:], qkp[:])
                    it2 = it % NIT_S
                    esl = slice(s_cur * 64, (s_cur + 1) * 64)
                    for s4 in range(4):
                        first = (it2 == 0 and grp == 0 and s4 == 0)
                        last = (it2 == NIT_S - 1 and grp == NSUB // 4 - 1 and s4 == 3)
                        nc.tensor.matmul(ep[:, esl], qk_sb[:, s4 * 128:s4 * 128 + 64],
                                         qk_sb[:, s4 * 128 + 64:s4 * 128 + 128],
                                         start=first, stop=last)

                if it % NIT_S == NIT_S - 1:
                    attT2_s = sample_epilogue(s_cur)
                    sample_out_phase(s_cur, attT2_s)

    nc.compile()
    return nc


def _get_nc():
    if 'nc' not in _CACHE:
        _CACHE['nc'] = _build_bass()
    return _CACHE['nc']


def _split16(a):
    h = a.astype(np.float16)
    l = (a - h.astype(np.float32)).astype(np.float16)
    return h, l


def kernel(rgb, hsv, lab, Wq, bq, Wk, bk, Wv, bv):
    from concourse.bass_utils import run_bass_kernel_spmd

    nc = _get_nc()

    rgb = np.asarray(rgb, dtype=np.float32)
    hsv = np.asarray(hsv, dtype=np.float32)
    lab = np.asarray(lab, dtype=np.float32)
    Wq = np.asarray(Wq, dtype=np.float32)
    Wk = np.asarray(Wk, dtype=np.float32)
    Wv = np.asarray(Wv, dtype=np.float32)
    bq = np.asarray(bq, dtype=np.float32)
    bk = np.asarray(bk, dtype=np.float32)
    bv = np.asarray(bv, dtype=np.float32)

    # weight prep: [192ch + ones-row, outs] with bias row, fp16 hi/lo
    wqk = np.concatenate([Wq.T, Wk.T], axis=1)          # [192, 128]
    bqk = np.concatenate([bq, bk])                      # [128]
    wqk_aug = np.vstack([wqk, bqk[None, :]])            # [193, 128]
    wqkh, wqkl = _split16(wqk_aug)
    wv_aug = np.vstack([Wv.T, bv[None, :]])             # [193, 64]
    wvh, _ = _split16(wv_aug)

    shared = {
        "wqkh0": wqkh[0:128], "wqkh1": wqkh[128:193],
        "wqkl0": wqkl[0:128], "wqkl1": wqkl[128:193],
        "wvh0": wvh[0:128], "wvh1": wvh[128:193],
        "ident": np.eye(64, dtype=np.float32),
    }

    # core c takes the c-th 1/8 of every sample's pixels, sample-major in
    # the free dim: [64, 4*8192]
    NPS = N // NCORES  # 8192 px per (core, sample)
    rgb_f = rgb.reshape(B, C, N)
    hsv_f = hsv.reshape(B, C, N)
    lab_f = lab.reshape(B, C, N)
    in_maps = []
    for c in range(NCORES):
        ps = slice(c * NPS, (c + 1) * NPS)
        in_maps.append({
            "x_rgb": np.ascontiguousarray(
                np.concatenate([rgb_f[s, :, ps] for s in range(B)], axis=1)),
            "x_hsv": np.ascontiguousarray(
                np.concatenate([hsv_f[s, :, ps] for s in range(B)], axis=1)),
            "x_lab": np.ascontiguousarray(
                np.concatenate([lab_f[s, :, ps] for s in range(B)], axis=1)),
            **shared,
        })

    res = run_bass_kernel_spmd(nc, in_maps, core_ids=list(range(NCORES)),
                               **_CACHE.get('run_kwargs', {}))
    _CACHE['last_results'] = res
    _CACHE['last_in_maps'] = in_maps

    out = np.empty((B, C, N), dtype=np.float32)
    for c in range(NCORES):
        ps = slice(c * NPS, (c + 1) * NPS)
        o = res.results[c]["out"]  # [64, 4*8192] sample-major
        for s in range(B):
            out[s, :, ps] = o[:, s * NPS:(s + 1) * NPS]
    return out.reshape(B, C, H, W)
